# revision 26
# baseline (speedup 1.0000x reference)
"""FLARetNet Trainium2 kernel: 8-core SPMD, batch x head-group sharding.

Each core handles one batch (B=2 -> 4 cores per batch) and 4 of 16 heads.
Per core: qkvg projections (fp16 matmuls), neox RoPE, RetNet chunked
retention scan (chunk=256), fused RMSNorm + swish gate, output projection
(partial sum over its heads). Host sums the 4 partials per batch.

fp16 matmuls run at full PE rate with fast weight load; the recurrent
state S and its o_inter matmul stay fp32r for precision.
"""
import numpy as np
import ml_dtypes

import concourse.bass as bass
import concourse.mybir as mybir
import concourse.tile as tile
import concourse.bacc as bacc
from concourse.bass_utils import run_bass_kernel_spmd

F32 = mybir.dt.float32
F32R = mybir.dt.float32r
BF16 = mybir.dt.float16
AF = mybir.ActivationFunctionType
BF = np.float16

B, T, D, H = 2, 4096, 1024, 16
DK, DV = 64, 128
C = 256            # attention chunk length (math-equivalent for any C)
PT = 512           # projection token-tile
NCH = T // C       # 16 chunks
HPC = 4            # heads per core
NCORES = 8

_cache = {}


def _build_program():
    nc = bacc.Bacc("TRN2", target_bir_lowering=False, debug=False)

    XT = nc.dram_tensor("XT", [D, T], BF16, kind="ExternalInput")
    WQ = nc.dram_tensor("WQ", [128, 8, 256], BF16, kind="ExternalInput")
    WK = nc.dram_tensor("WK", [128, 8, 256], BF16, kind="ExternalInput")
    WV = nc.dram_tensor("WV", [128, 8, 512], BF16, kind="ExternalInput")
    WG = nc.dram_tensor("WG", [128, 8, 512], BF16, kind="ExternalInput")
    WO = nc.dram_tensor("WO", [128, 4, 1024], BF16, kind="ExternalInput")
    CS = nc.dram_tensor("CS", [128, 4, T], BF16, kind="ExternalInput")
    DQT = nc.dram_tensor("DQT", [128, 2, C], BF16, kind="ExternalInput")
    DKVT = nc.dram_tensor("DKVT", [128, 2, C], F32, kind="ExternalInput")
    DMT = nc.dram_tensor("DMT", [128, HPC, 2, C], F32, kind="ExternalInput")
    GCV = nc.dram_tensor("GCV", [128, 2], F32, kind="ExternalInput")
    GW = nc.dram_tensor("GW", [128, 1], F32, kind="ExternalInput")
    ONES = nc.dram_tensor("ONES", [128, 1], BF16, kind="ExternalInput")
    ONESR = nc.dram_tensor("ONESR", [1, 128], BF16, kind="ExternalInput")
    IDENT = nc.dram_tensor("IDENT", [128, 128], BF16, kind="ExternalInput")
    ZS = nc.dram_tensor("ZS", [128, 2, C], BF16, kind="ExternalInput")

    OUT = nc.dram_tensor("OUT", [T, D], F32, kind="ExternalOutput")

    with tile.TileContext(nc) as tc:
        with tc.tile_pool(name="singles", bufs=1) as singles, \
             tc.tile_pool(name="xt", bufs=2) as xt_pool, \
             tc.tile_pool(name="tab", bufs=2) as tab_pool, \
             tc.tile_pool(name="rope", bufs=2) as rope_pool, \
             tc.tile_pool(name="qk", bufs=2) as qk_pool, \
             tc.tile_pool(name="vsb", bufs=2) as v_pool, \
             tc.tile_pool(name="asb", bufs=3) as a_pool, \
             tc.tile_pool(name="gat", bufs=2) as g_pool, \
             tc.tile_pool(name="nrm", bufs=2) as nrm_pool, \
             tc.tile_pool(name="og", bufs=4) as og_pool, \
             tc.tile_pool(name="osb", bufs=3) as out_pool, \
             tc.tile_pool(name="ps_proj", bufs=3, space="PSUM") as ps_proj, \
             tc.tile_pool(name="ps_small", bufs=3, space="PSUM") as ps_small, \
             tc.tile_pool(name="ps_o", bufs=2, space="PSUM") as ps_o:

            # ---- resident weights/tables ----
            wq = singles.tile([128, 8, 256], BF16)
            wk = singles.tile([128, 8, 256], BF16)
            wv = singles.tile([128, 8, 512], BF16)
            wg = singles.tile([128, 8, 512], BF16)
            wo = singles.tile([128, 4, 1024], BF16)
            nc.gpsimd.dma_start(out=wq, in_=WQ[:, :, :])
            nc.gpsimd.dma_start(out=wk, in_=WK[:, :, :])
            nc.gpsimd.dma_start(out=wv, in_=WV[:, :, :])
            nc.gpsimd.dma_start(out=wg, in_=WG[:, :, :])
            nc.gpsimd.dma_start(out=wo, in_=WO[:, :, :])

            dqt = singles.tile([128, 2, C], BF16)
            dkvt = singles.tile([128, 2, C], F32)
            dmt = singles.tile([128, HPC, 2, C], F32)
            gcv = singles.tile([128, 2], F32)
            gwt = singles.tile([128, 1], F32)
            ones = singles.tile([128, 1], BF16)
            identmm = singles.tile([128, 128], BF16)
            nc.gpsimd.dma_start(out=identmm, in_=IDENT[:, :])
            nc.gpsimd.dma_start(out=dqt, in_=DQT[:, :, :])
            nc.gpsimd.dma_start(out=dkvt, in_=DKVT[:, :, :])
            nc.gpsimd.dma_start(out=dmt, in_=DMT[:, :, :, :])
            nc.gpsimd.dma_start(out=gcv, in_=GCV[:, :])
            nc.gpsimd.dma_start(out=gwt, in_=GW[:, :])
            nc.gpsimd.dma_start(out=ones, in_=ONES[:, :])
            onesr = singles.tile([1, 128], BF16)
            nc.gpsimd.dma_start(out=onesr, in_=ONESR[:, :])

            # persistent recurrent state, packed per head-pair:
            # S_sb[:, p, :]: rows (h%2)*64..+64, cols (h%2)*128..+128 hold S_h
            s_sb = singles.tile([128, 2, C], BF16)
            nc.gpsimd.dma_start(out=s_sb, in_=ZS[:, :, :])

            epsb = singles.tile([1, 1], F32)
            nc.vector.memset(epsb, 1e-5)

            def emit_wo(og_pair_list, oc0):
                for tb in range(2):
                    for nn in range(2):
                        out_ps = ps_small.tile([128, 512], F32, tag="small",
                                               name=f"wo{oc0}_{tb}_{nn}")
                        for h in range(HPC):
                            nc.tensor.matmul(
                                out_ps,
                                lhsT=og_pair_list[h // 2][
                                    :, h % 2, tb * 128:(tb + 1) * 128],
                                rhs=wo[:, h, nn * 512:(nn + 1) * 512],
                                start=(h == 0), stop=(h == HPC - 1))
                        out_sb = out_pool.tile([128, 512], F32, tag="outsb",
                                               name=f"wos{oc0}_{tb}_{nn}")
                        nc.scalar.copy(out_sb, out_ps)
                        nc.sync.dma_start(
                            out=OUT[oc0 + tb * 128:oc0 + (tb + 1) * 128,
                                    nn * 512:(nn + 1) * 512],
                            in_=out_sb)

            pending_wo = None

            for pt in range(T // PT):
                p0 = pt * PT

                xt = xt_pool.tile([128, 8, PT], BF16, tag="xt")
                xt_src = XT.rearrange("(db p) t -> p db t", p=128)
                nc.sync.dma_start(out=xt[:, 0:4, :],
                                  in_=xt_src[:, 0:4, p0:p0 + PT])
                nc.sync.dma_start(out=xt[:, 4:8, :],
                                  in_=xt_src[:, 4:8, p0:p0 + PT])

                cs = tab_pool.tile([128, 4, PT], BF16, tag="cs")
                nc.sync.dma_start(out=cs, in_=CS[:, :, p0:p0 + PT])
                cos = cs[:, 0:2, :]
                sin = cs[:, 2:4, :]

                # ---- projections over PT tokens ----
                # q, k feature-major [128(dim%128), blk, tok] + RoPE fused
                def proj_rope(w, tag):
                    out = qk_pool.tile([128, 2, PT], BF16, tag=tag,
                                       name=f"{tag}{pt}")
                    for m in range(2):
                        pps = ps_proj.tile([128, PT], F32, tag="proj",
                                           name=f"{tag}ps{pt}_{m}")
                        for db in range(8):
                            nc.tensor.matmul(
                                pps, lhsT=w[:, db, m * 128:(m + 1) * 128],
                                rhs=xt[:, db, :],
                                start=(db == 0), stop=(db == 7))
                        tcos = rope_pool.tile([128, PT], F32, tag="tcos")
                        tsin = rope_pool.tile([128, PT], BF16, tag="tsin")
                        rot = rope_pool.tile([128, PT], BF16, tag="rot")
                        nc.vector.tensor_mul(tcos, pps, cos[:, m, :])
                        nc.vector.tensor_mul(tsin, pps, sin[:, m, :])
                        for g0 in (0, 64):
                            nc.gpsimd.dma_start(out=rot[g0:g0 + 32, :],
                                                in_=tsin[g0 + 32:g0 + 64, :])
                            nc.gpsimd.dma_start(out=rot[g0 + 32:g0 + 64, :],
                                                in_=tsin[g0:g0 + 32, :])
                        nc.vector.tensor_add(out[:, m, :], tcos, rot)
                    return out

                q_sb = proj_rope(wq, "q")   # [128, 2, PT] fp16
                k_sb = proj_rope(wk, "k")

                qdq = qk_pool.tile([128, 2, PT], BF16, tag="qdq")
                nc.vector.tensor_mul(
                    qdq.rearrange('p a (c b) -> p a c b', b=C),
                    q_sb.rearrange('p a (c b) -> p a c b', b=C),
                    dqt[:, :, None, :].broadcast_to([128, 2, PT // C, C]))

                # v token-major [128(tok%128), tb, dim]
                v_sb = v_pool.tile([128, 4, 512], BF16, tag="v")
                for tb in range(4):
                    v_ps = ps_proj.tile([128, 512], F32, tag="proj",
                                        name=f"vps{pt}_{tb}")
                    for db in range(8):
                        nc.tensor.matmul(
                            v_ps,
                            lhsT=xt[:, db, tb * 128:(tb + 1) * 128],
                            rhs=wv[:, db, :],
                            start=(db == 0), stop=(db == 7))
                    nc.scalar.copy(v_sb[:, tb, :], v_ps)

                # g feature-major per head-block -> silu
                gsil = g_pool.tile([128, 4, PT], F32, tag="gsil")
                for m in range(4):
                    g_ps = ps_proj.tile([128, PT], F32, tag="proj",
                                        name=f"gps{pt}_{m}")
                    for db in range(8):
                        nc.tensor.matmul(
                            g_ps, lhsT=wg[:, db, m * 128:(m + 1) * 128],
                            rhs=xt[:, db, :],
                            start=(db == 0), stop=(db == 7))
                    nc.scalar.activation(gsil[:, m, :], g_ps, AF.Silu)

                # ---- per 256-chunk attention ----
                for cc in range(PT // C):
                    ch = pt * (PT // C) + cc
                    c0 = ch * C
                    qs = q_sb[:, :, cc * C:(cc + 1) * C]
                    ks = k_sb[:, :, cc * C:(cc + 1) * C]
                    qd = qdq[:, :, cc * C:(cc + 1) * C]
                    vtb0 = cc * 2

                    # k token-major + dkv scaling
                    ktm_ps = ps_small.tile([128, 2, C], BF16, tag="small",
                                           name=f"ktm{ch}")
                    for tb in range(2):
                        for b in range(2):
                            nc.tensor.transpose(
                                ktm_ps[:, tb, b * 128:(b + 1) * 128],
                                ks[:, b, tb * 128:(tb + 1) * 128],
                                identmm)
                    kdkv = qk_pool.tile([128, 2, C], BF16, tag="kdkv")
                    nc.vector.tensor_mul(kdkv, ktm_ps, dkvt)

                    o_ps_pairs = [ps_o.tile([128, 2, C], F32, tag="o",
                                            name=f"o_ps{ch}_{i}")
                                  for i in range(2)]
                    a_sbs = [None] * HPC

                    def emit_at(h):
                        blk, pb = h // 2, (h % 2) * 64
                        at_ps = ps_small.tile([128, 2, C], F32, tag="small",
                                              name=f"at{ch}_{h}")
                        for jb in range(2):
                            nc.tensor.matmul(
                                at_ps[:, jb, :],
                                lhsT=ks[pb:pb + 64, blk,
                                        jb * 128:(jb + 1) * 128],
                                rhs=qs[pb:pb + 64, blk, :],
                                start=True, stop=True)
                        a_sb = a_pool.tile([128, 2, C], BF16, tag="a",
                                           name=f"a{ch}_{h}")
                        nc.vector.tensor_mul(a_sb, at_ps, dmt[:, h, :, :])
                        a_sbs[h] = a_sb

                    def emit_o(h):
                        p, hh = h // 2, h % 2
                        blk, pb = h // 2, (h % 2) * 64
                        o_slice = o_ps_pairs[p][:, hh, :]
                        nc.tensor.matmul(
                            o_slice,
                            lhsT=s_sb[hh * 64:hh * 64 + 64, p,
                                      hh * 128:(hh + 1) * 128],
                            rhs=qd[pb:pb + 64, blk, :],
                            start=True, stop=False)
                        for jb in range(2):
                            nc.tensor.matmul(
                                o_slice,
                                lhsT=v_sb[:, vtb0 + jb, h * 128:(h + 1) * 128],
                                rhs=a_sbs[h][:, jb, :],
                                start=False, stop=(jb == 1))

                    emit_at(0)
                    for h in range(1, HPC):
                        emit_at(h)
                        emit_o(h - 1)

                    # deferred output projection of previous chunk: gives the
                    # previous norm chain time to finish off the PE critical path
                    if pending_wo is not None:
                        emit_wo(*pending_wo)
                        pending_wo = None

                    emit_o(HPC - 1)

                    # state update (packed per pair)
                    ds_ps = ps_small.tile([128, 2, C], F32, tag="small",
                                          name=f"ds{ch}")
                    for p in range(2):
                        for jb in range(2):
                            nc.tensor.matmul(
                                ds_ps[:, p, :],
                                lhsT=kdkv[:, jb, p * 128:(p + 1) * 128],
                                rhs=v_sb[:, vtb0 + jb, p * 256:(p + 1) * 256],
                                start=(jb == 0), stop=(jb == 1))
                    for p in range(2):
                        nc.vector.scalar_tensor_tensor(
                            out=s_sb[:, p, :],
                            in0=s_sb[:, p, :],
                            scalar=gcv[:, p:p + 1],
                            in1=ds_ps[:, p, :],
                            op0=mybir.AluOpType.mult,
                            op1=mybir.AluOpType.add)

                    # norm + gate per pair
                    og_pairs = []
                    for p in range(2):
                        o_ps = o_ps_pairs[p]
                        o_flat = o_ps.rearrange('p a b -> p (a b)')
                        o2 = nrm_pool.tile([128, 512], BF16, tag="o2",
                                           name=f"o2{ch}_{p}")
                        nc.scalar.activation(o2, o_flat, AF.Square)
                        mean_ps = ps_small.tile([1, 512], F32, tag="small",
                                                name=f"mean{ch}_{p}")
                        nc.tensor.matmul(mean_ps, lhsT=ones, rhs=o2,
                                         start=True, stop=True)
                        rsq1 = nrm_pool.tile([1, 512], F32, tag="rsq1",
                                             name=f"rsq{ch}_{p}")
                        nc.scalar.activation(rsq1, mean_ps,
                                             AF.Abs_reciprocal_sqrt,
                                             bias=epsb, scale=1.0 / DV)
                        bc = nrm_pool.tile([128, 512], F32, tag="bc",
                                           name=f"bcb{ch}_{p}")
                        nc.gpsimd.partition_broadcast(bc, rsq1)
                        onrm = nrm_pool.tile([128, 512], F32, tag="onrm",
                                             name=f"onrm{ch}_{p}")
                        nc.vector.tensor_mul(onrm, o_flat, bc)
                        og = og_pool.tile([128, 2, C], BF16, tag="og",
                                          name=f"og{ch}_{p}")
                        gs = gsil[:, p * 2:(p + 1) * 2, cc * C:(cc + 1) * C]
                        nc.vector.scalar_tensor_tensor(
                            out=og,
                            in0=onrm.rearrange('p (a b) -> p a b', a=2),
                            scalar=gwt, in1=gs,
                            op0=mybir.AluOpType.mult,
                            op1=mybir.AluOpType.mult)
                        og_pairs.append(og)

                    if ch == NCH - 1:
                        emit_wo(og_pairs, c0)
                    else:
                        pending_wo = (og_pairs, c0)

    nc.finalize()
    return nc


def _host_tables(heads):
    """Per-core constant tables for a 4-head slice."""
    gam = (1.0 - 2.0 ** (-5.0 - np.arange(H, dtype=np.float64)))[heads]  # [4]
    i_idx = np.arange(C, dtype=np.float64)

    # rope tables, feature-major [128, 2, T]
    inv = 10000.0 ** (-np.arange(0, DK, 2, dtype=np.float64) / DK)  # [32]
    t_idx = np.arange(T, dtype=np.float64)
    ang = np.outer(t_idx, inv)                      # [T, 32]
    cos_t, sin_t = np.cos(ang), np.sin(ang)         # [T, 32]
    COSt = np.empty((128, 2, T), np.float32)
    SINt = np.empty((128, 2, T), np.float32)
    for b in range(2):
        for p in range(128):
            d = b * 128 + p
            dd = d % 64
            idx = dd % 32
            sign = 1.0 if dd < 32 else -1.0
            COSt[p, b, :] = cos_t[:, idx]
            SINt[p, b, :] = sign * sin_t[:, idx]

    # decay tables (chunk-invariant), feature-major [128, 2, C]
    DQt = np.empty((128, 2, C), np.float32)
    for b in range(2):
        for p in range(128):
            h = (b * 128 + p) // 64
            DQt[p, b, :] = gam[h] ** (i_idx + 1.0)
    # dkv token-major [128(j%128), 2(jb), C(dim col)]
    DKVt = np.empty((128, 2, C), np.float32)
    for jb in range(2):
        j = jb * 128 + np.arange(128, dtype=np.float64)
        for hcol in range(4):
            DKVt[:, jb, hcol * 64:(hcol + 1) * 64] = (
                gam[hcol] ** (C - 1.0 - j))[:, None]
    # Dmat^T [128(j%128), h, jb, C(i)]
    DMTt = np.zeros((128, HPC, 2, C), np.float32)
    for h in range(HPC):
        for jb in range(2):
            j = (jb * 128 + np.arange(128, dtype=np.float64))[:, None]
            rel = i_idx[None, :] - j
            DMTt[:, h, jb, :] = np.where(rel >= 0.0, gam[h] ** np.maximum(rel, 0.0), 0.0)
    # gamma^C per state-pair row
    GCVt = np.empty((128, 2), np.float32)
    for p in range(2):
        GCVt[0:64, p] = gam[2 * p] ** C
        GCVt[64:128, p] = gam[2 * p + 1] ** C
    return COSt, SINt, DQt, DKVt, DMTt, GCVt


def _prepare_inputs(x, Wq, Wk, Wv, Wg, Wo, g_norm_w):
    x = np.asarray(x, np.float32)
    Wq = np.asarray(Wq, np.float32) * (DK ** -0.5)
    Wk = np.asarray(Wk, np.float32)
    Wv = np.asarray(Wv, np.float32)
    Wg = np.asarray(Wg, np.float32)
    Wo = np.asarray(Wo, np.float32)
    gw = np.asarray(g_norm_w, np.float32)

    in_maps = []
    for core in range(NCORES):
        b = core // 4
        hg = core % 4
        heads = np.arange(4 * hg, 4 * hg + 4)
        qk_cols = np.concatenate([np.arange(h * DK, (h + 1) * DK) for h in heads])
        vg_cols = np.concatenate([np.arange(h * DV, (h + 1) * DV) for h in heads])

        XTc = np.ascontiguousarray(x[b].T).astype(BF)
        WQc = np.ascontiguousarray(
            Wq[:, qk_cols].reshape(8, 128, 256).transpose(1, 0, 2)).astype(BF)
        WKc = np.ascontiguousarray(
            Wk[:, qk_cols].reshape(8, 128, 256).transpose(1, 0, 2)).astype(BF)
        WVc = np.ascontiguousarray(
            Wv[:, vg_cols].reshape(8, 128, 512).transpose(1, 0, 2)).astype(BF)
        WGc = np.ascontiguousarray(
            Wg[:, vg_cols].reshape(8, 128, 512).transpose(1, 0, 2)).astype(BF)
        WOc = np.ascontiguousarray(
            Wo[vg_cols, :].reshape(4, 128, 1024).transpose(1, 0, 2)).astype(BF)

        COSt, SINt, DQt, DKVt, DMTt, GCVt = _host_tables(heads)
        CSt = np.concatenate([COSt, SINt], axis=1).astype(BF)

        in_maps.append({
            "XT": XTc, "WQ": WQc, "WK": WKc, "WV": WVc, "WG": WGc, "WO": WOc,
            "CS": CSt, "DQT": DQt.astype(BF), "DKVT": DKVt, "DMT": DMTt,
            "GCV": GCVt, "GW": np.ascontiguousarray(gw.reshape(128, 1)),
            "ONES": np.ones((128, 1), BF),
            "ONESR": np.ones((1, 128), BF),
            "IDENT": np.eye(128, dtype=BF),
            "ZS": np.zeros((128, 2, C), BF),
        })
    return in_maps


def _run(in_maps, **kw):
    if "nc" not in _cache:
        _cache["nc"] = _build_program()
    return run_bass_kernel_spmd(_cache["nc"], in_maps,
                                core_ids=list(range(NCORES)), **kw)


def kernel(x, Wq, Wk, Wv, Wg, Wo, g_norm_w):
    in_maps = _prepare_inputs(x, Wq, Wk, Wv, Wg, Wo, g_norm_w)
    res = _run(in_maps)
    out = np.zeros((B, T, D), np.float32)
    for core in range(NCORES):
        out[core // 4] += res.results[core]["OUT"]
    return out


# revision 27
# speedup vs baseline: 1.0221x; 1.0221x over previous
"""FLARetNet Trainium2 kernel: 8-core SPMD, batch x head-group sharding.

Each core handles one batch (B=2 -> 4 cores per batch) and 4 of 16 heads.
Per core: qkvg projections (fp16 matmuls), neox RoPE, RetNet chunked
retention scan (chunk=256), fused RMSNorm + swish gate, output projection
(partial sum over its heads). Host sums the 4 partials per batch.

fp16 matmuls run at full PE rate with fast weight load; the recurrent
state S and its o_inter matmul stay fp32r for precision.
"""
import numpy as np
import ml_dtypes

import concourse.bass as bass
import concourse.mybir as mybir
import concourse.tile as tile
import concourse.bacc as bacc
from concourse.bass_utils import run_bass_kernel_spmd

F32 = mybir.dt.float32
F32R = mybir.dt.float32r
BF16 = mybir.dt.float16
AF = mybir.ActivationFunctionType
BF = np.float16

B, T, D, H = 2, 4096, 1024, 16
DK, DV = 64, 128
C = 256            # attention chunk length (math-equivalent for any C)
PT = 512           # projection token-tile
NCH = T // C       # 16 chunks
HPC = 4            # heads per core
NCORES = 8

_cache = {}


def _build_program():
    nc = bacc.Bacc("TRN2", target_bir_lowering=False, debug=False)

    XT = nc.dram_tensor("XT", [D, T], BF16, kind="ExternalInput")
    WQ = nc.dram_tensor("WQ", [128, 8, 256], BF16, kind="ExternalInput")
    WK = nc.dram_tensor("WK", [128, 8, 256], BF16, kind="ExternalInput")
    WV = nc.dram_tensor("WV", [128, 8, 512], BF16, kind="ExternalInput")
    WG = nc.dram_tensor("WG", [128, 8, 512], BF16, kind="ExternalInput")
    WO = nc.dram_tensor("WO", [128, 4, 1024], BF16, kind="ExternalInput")
    CS = nc.dram_tensor("CS", [128, 4, T], BF16, kind="ExternalInput")
    DQT = nc.dram_tensor("DQT", [128, 2, C], BF16, kind="ExternalInput")
    DKVT = nc.dram_tensor("DKVT", [128, 2, C], F32, kind="ExternalInput")
    DMT = nc.dram_tensor("DMT", [128, HPC, 2, C], F32, kind="ExternalInput")
    GCV = nc.dram_tensor("GCV", [128, 2], F32, kind="ExternalInput")
    GW = nc.dram_tensor("GW", [128, 1], F32, kind="ExternalInput")
    ONES = nc.dram_tensor("ONES", [128, 1], BF16, kind="ExternalInput")
    ONESR = nc.dram_tensor("ONESR", [1, 128], BF16, kind="ExternalInput")
    IDENT = nc.dram_tensor("IDENT", [128, 128], BF16, kind="ExternalInput")
    ZS = nc.dram_tensor("ZS", [128, 2, C], BF16, kind="ExternalInput")

    OUT = nc.dram_tensor("OUT", [T, D], F32, kind="ExternalOutput")

    with tile.TileContext(nc) as tc:
        with tc.tile_pool(name="singles", bufs=1) as singles, \
             tc.tile_pool(name="xt", bufs=2) as xt_pool, \
             tc.tile_pool(name="tab", bufs=2) as tab_pool, \
             tc.tile_pool(name="rope", bufs=2) as rope_pool, \
             tc.tile_pool(name="qk", bufs=2) as qk_pool, \
             tc.tile_pool(name="vsb", bufs=2) as v_pool, \
             tc.tile_pool(name="asb", bufs=3) as a_pool, \
             tc.tile_pool(name="gat", bufs=2) as g_pool, \
             tc.tile_pool(name="nrm", bufs=2) as nrm_pool, \
             tc.tile_pool(name="og", bufs=4) as og_pool, \
             tc.tile_pool(name="osb", bufs=3) as out_pool, \
             tc.tile_pool(name="ps_proj", bufs=3, space="PSUM") as ps_proj, \
             tc.tile_pool(name="ps_small", bufs=3, space="PSUM") as ps_small, \
             tc.tile_pool(name="ps_o", bufs=2, space="PSUM") as ps_o:

            # ---- resident weights/tables ----
            wq = singles.tile([128, 8, 256], BF16)
            wk = singles.tile([128, 8, 256], BF16)
            wv = singles.tile([128, 8, 512], BF16)
            wg = singles.tile([128, 8, 512], BF16)
            wo = singles.tile([128, 4, 1024], BF16)
            nc.gpsimd.dma_start(out=wq, in_=WQ[:, :, :])
            nc.gpsimd.dma_start(out=wk, in_=WK[:, :, :])
            nc.gpsimd.dma_start(out=wv, in_=WV[:, :, :])
            nc.gpsimd.dma_start(out=wg, in_=WG[:, :, :])
            nc.gpsimd.dma_start(out=wo, in_=WO[:, :, :])

            dqt = singles.tile([128, 2, C], BF16)
            dkvt = singles.tile([128, 2, C], F32)
            dmt = singles.tile([128, HPC, 2, C], F32)
            gcv = singles.tile([128, 2], F32)
            gwt = singles.tile([128, 1], F32)
            ones = singles.tile([128, 1], BF16)
            identmm = singles.tile([128, 128], BF16)
            nc.gpsimd.dma_start(out=identmm, in_=IDENT[:, :])
            nc.gpsimd.dma_start(out=dqt, in_=DQT[:, :, :])
            nc.gpsimd.dma_start(out=dkvt, in_=DKVT[:, :, :])
            nc.gpsimd.dma_start(out=dmt, in_=DMT[:, :, :, :])
            nc.gpsimd.dma_start(out=gcv, in_=GCV[:, :])
            nc.gpsimd.dma_start(out=gwt, in_=GW[:, :])
            nc.gpsimd.dma_start(out=ones, in_=ONES[:, :])
            onesr = singles.tile([1, 128], BF16)
            nc.gpsimd.dma_start(out=onesr, in_=ONESR[:, :])

            # persistent recurrent state, packed per head-pair:
            # S_sb[:, p, :]: rows (h%2)*64..+64, cols (h%2)*128..+128 hold S_h
            s_sb = singles.tile([128, 2, C], BF16)
            nc.gpsimd.dma_start(out=s_sb, in_=ZS[:, :, :])

            epsb = singles.tile([1, 1], F32)
            nc.vector.memset(epsb, 1e-5)

            def emit_wo(og_pair_list, oc0):
                for tb in range(2):
                    for nn in range(2):
                        out_ps = ps_small.tile([128, 512], F32, tag="small",
                                               name=f"wo{oc0}_{tb}_{nn}")
                        for h in range(HPC):
                            nc.tensor.matmul(
                                out_ps,
                                lhsT=og_pair_list[h // 2][
                                    :, h % 2, tb * 128:(tb + 1) * 128],
                                rhs=wo[:, h, nn * 512:(nn + 1) * 512],
                                start=(h == 0), stop=(h == HPC - 1))
                        out_sb = out_pool.tile([128, 512], F32, tag="outsb",
                                               name=f"wos{oc0}_{tb}_{nn}")
                        nc.scalar.copy(out_sb, out_ps)
                        nc.sync.dma_start(
                            out=OUT[oc0 + tb * 128:oc0 + (tb + 1) * 128,
                                    nn * 512:(nn + 1) * 512],
                            in_=out_sb)

            pending_wo = None

            for pt in range(T // PT):
                p0 = pt * PT

                xt = xt_pool.tile([128, 8, PT], BF16, tag="xt")
                xt_src = XT.rearrange("(db p) t -> p db t", p=128)
                nc.sync.dma_start(out=xt[:, 0:4, :],
                                  in_=xt_src[:, 0:4, p0:p0 + PT])
                nc.sync.dma_start(out=xt[:, 4:8, :],
                                  in_=xt_src[:, 4:8, p0:p0 + PT])

                cs = tab_pool.tile([128, 4, PT], BF16, tag="cs")
                nc.sync.dma_start(out=cs, in_=CS[:, :, p0:p0 + PT])
                cos = cs[:, 0:2, :]
                sin = cs[:, 2:4, :]

                # ---- projections over PT tokens ----
                # q, k feature-major [128(dim%128), blk, tok] + RoPE fused
                def proj_rope(w, tag):
                    out = qk_pool.tile([128, 2, PT], BF16, tag=tag,
                                       name=f"{tag}{pt}")
                    for m in range(2):
                        pps = ps_proj.tile([128, PT], F32, tag="proj",
                                           name=f"{tag}ps{pt}_{m}")
                        for db in range(8):
                            nc.tensor.matmul(
                                pps, lhsT=w[:, db, m * 128:(m + 1) * 128],
                                rhs=xt[:, db, :],
                                start=(db == 0), stop=(db == 7))
                        tcos = rope_pool.tile([128, PT], F32, tag="tcos")
                        tsin = rope_pool.tile([128, PT], BF16, tag="tsin")
                        rot = rope_pool.tile([128, PT], BF16, tag="rot")
                        nc.vector.tensor_mul(tcos, pps, cos[:, m, :])
                        nc.vector.tensor_mul(tsin, pps, sin[:, m, :])
                        for g0 in (0, 64):
                            nc.gpsimd.dma_start(out=rot[g0:g0 + 32, :],
                                                in_=tsin[g0 + 32:g0 + 64, :])
                            nc.gpsimd.dma_start(out=rot[g0 + 32:g0 + 64, :],
                                                in_=tsin[g0:g0 + 32, :])
                        nc.vector.tensor_add(out[:, m, :], tcos, rot)
                    return out

                q_sb = proj_rope(wq, "q")   # [128, 2, PT] fp16
                k_sb = proj_rope(wk, "k")

                qdq = qk_pool.tile([128, 2, PT], BF16, tag="qdq")
                nc.vector.tensor_mul(
                    qdq.rearrange('p a (c b) -> p a c b', b=C),
                    q_sb.rearrange('p a (c b) -> p a c b', b=C),
                    dqt[:, :, None, :].broadcast_to([128, 2, PT // C, C]))

                # v token-major [128(tok%128), tb, dim]
                v_sb = v_pool.tile([128, 4, 512], BF16, tag="v")
                for tb in range(4):
                    v_ps = ps_proj.tile([128, 512], F32, tag="proj",
                                        name=f"vps{pt}_{tb}")
                    for db in range(8):
                        nc.tensor.matmul(
                            v_ps,
                            lhsT=xt[:, db, tb * 128:(tb + 1) * 128],
                            rhs=wv[:, db, :],
                            start=(db == 0), stop=(db == 7))
                    nc.scalar.copy(v_sb[:, tb, :], v_ps)

                # g feature-major per head-block -> silu
                gsil = g_pool.tile([128, 4, PT], F32, tag="gsil")
                for m in range(4):
                    g_ps = ps_proj.tile([128, PT], F32, tag="proj",
                                        name=f"gps{pt}_{m}")
                    for db in range(8):
                        nc.tensor.matmul(
                            g_ps, lhsT=wg[:, db, m * 128:(m + 1) * 128],
                            rhs=xt[:, db, :],
                            start=(db == 0), stop=(db == 7))
                    nc.scalar.activation(gsil[:, m, :], g_ps, AF.Silu)

                # ---- per 256-chunk attention ----
                for cc in range(PT // C):
                    ch = pt * (PT // C) + cc
                    c0 = ch * C
                    qs = q_sb[:, :, cc * C:(cc + 1) * C]
                    ks = k_sb[:, :, cc * C:(cc + 1) * C]
                    qd = qdq[:, :, cc * C:(cc + 1) * C]
                    vtb0 = cc * 2

                    # k token-major + dkv scaling
                    ktm_ps = ps_small.tile([128, 2, C], BF16, tag="small",
                                           name=f"ktm{ch}")
                    for tb in range(2):
                        for b in range(2):
                            nc.tensor.transpose(
                                ktm_ps[:, tb, b * 128:(b + 1) * 128],
                                ks[:, b, tb * 128:(tb + 1) * 128],
                                identmm)
                    kdkv = qk_pool.tile([128, 2, C], BF16, tag="kdkv")
                    nc.vector.tensor_mul(kdkv, ktm_ps, dkvt)

                    o_ps_pairs = [ps_o.tile([128, 2, C], F32, tag="o",
                                            name=f"o_ps{ch}_{i}")
                                  for i in range(2)]
                    a_sbs = [None] * HPC

                    def emit_at(h):
                        blk, pb = h // 2, (h % 2) * 64
                        at_ps = ps_small.tile([128, 2, C], F32, tag="small",
                                              name=f"at{ch}_{h}")
                        for jb in range(2):
                            nc.tensor.matmul(
                                at_ps[:, jb, :],
                                lhsT=ks[pb:pb + 64, blk,
                                        jb * 128:(jb + 1) * 128],
                                rhs=qs[pb:pb + 64, blk, :],
                                start=True, stop=True)
                        a_sb = a_pool.tile([128, 2, C], BF16, tag="a",
                                           name=f"a{ch}_{h}")
                        nc.vector.tensor_mul(a_sb, at_ps, dmt[:, h, :, :])
                        a_sbs[h] = a_sb

                    def emit_o(h):
                        p, hh = h // 2, h % 2
                        blk, pb = h // 2, (h % 2) * 64
                        o_slice = o_ps_pairs[p][:, hh, :]
                        nc.tensor.matmul(
                            o_slice,
                            lhsT=s_sb[hh * 64:hh * 64 + 64, p,
                                      hh * 128:(hh + 1) * 128],
                            rhs=qd[pb:pb + 64, blk, :],
                            start=True, stop=False)
                        for jb in range(2):
                            nc.tensor.matmul(
                                o_slice,
                                lhsT=v_sb[:, vtb0 + jb, h * 128:(h + 1) * 128],
                                rhs=a_sbs[h][:, jb, :],
                                start=False, stop=(jb == 1))

                    emit_at(0)
                    for h in range(1, HPC):
                        emit_at(h)
                        emit_o(h - 1)

                    # deferred output projection of previous chunk: gives the
                    # previous norm chain time to finish off the PE critical path
                    if pending_wo is not None:
                        emit_wo(*pending_wo)
                        pending_wo = None

                    emit_o(HPC - 1)

                    # state update (packed per pair)
                    ds_ps = ps_small.tile([128, 2, C], F32, tag="small",
                                          name=f"ds{ch}")
                    for p in range(2):
                        for jb in range(2):
                            nc.tensor.matmul(
                                ds_ps[:, p, :],
                                lhsT=kdkv[:, jb, p * 128:(p + 1) * 128],
                                rhs=v_sb[:, vtb0 + jb, p * 256:(p + 1) * 256],
                                start=(jb == 0), stop=(jb == 1))
                    for p in range(2):
                        nc.vector.scalar_tensor_tensor(
                            out=s_sb[:, p, :],
                            in0=s_sb[:, p, :],
                            scalar=gcv[:, p:p + 1],
                            in1=ds_ps[:, p, :],
                            op0=mybir.AluOpType.mult,
                            op1=mybir.AluOpType.add)

                    # norm + gate per pair
                    og_pairs = []
                    for p in range(2):
                        o_ps = o_ps_pairs[p]
                        o_flat = o_ps.rearrange('p a b -> p (a b)')
                        osc = nrm_pool.tile([128, 512], F32, tag="osc",
                                            name=f"osc{ch}_{p}")
                        nc.scalar.copy(osc, o_flat)
                        o2 = nrm_pool.tile([128, 512], BF16, tag="o2",
                                           name=f"o2{ch}_{p}")
                        nc.vector.tensor_mul(o2, osc, o_flat)
                        mean_ps = ps_small.tile([1, 512], F32, tag="small",
                                                name=f"mean{ch}_{p}")
                        nc.tensor.matmul(mean_ps, lhsT=ones, rhs=o2,
                                         start=True, stop=True)
                        rsq1 = nrm_pool.tile([1, 512], F32, tag="rsq1",
                                             name=f"rsq{ch}_{p}")
                        nc.scalar.activation(rsq1, mean_ps,
                                             AF.Abs_reciprocal_sqrt,
                                             bias=epsb, scale=1.0 / DV)
                        bc = nrm_pool.tile([128, 512], F32, tag="bc",
                                           name=f"bcb{ch}_{p}")
                        nc.gpsimd.partition_broadcast(bc, rsq1)
                        onrm = nrm_pool.tile([128, 512], F32, tag="onrm",
                                             name=f"onrm{ch}_{p}")
                        nc.vector.tensor_mul(onrm, osc, bc)
                        og = og_pool.tile([128, 2, C], BF16, tag="og",
                                          name=f"og{ch}_{p}")
                        gs = gsil[:, p * 2:(p + 1) * 2, cc * C:(cc + 1) * C]
                        nc.vector.scalar_tensor_tensor(
                            out=og,
                            in0=onrm.rearrange('p (a b) -> p a b', a=2),
                            scalar=gwt, in1=gs,
                            op0=mybir.AluOpType.mult,
                            op1=mybir.AluOpType.mult)
                        og_pairs.append(og)

                    if ch == NCH - 1:
                        emit_wo(og_pairs, c0)
                    else:
                        pending_wo = (og_pairs, c0)

    nc.finalize()
    return nc


def _host_tables(heads):
    """Per-core constant tables for a 4-head slice."""
    gam = (1.0 - 2.0 ** (-5.0 - np.arange(H, dtype=np.float64)))[heads]  # [4]
    i_idx = np.arange(C, dtype=np.float64)

    # rope tables, feature-major [128, 2, T]
    inv = 10000.0 ** (-np.arange(0, DK, 2, dtype=np.float64) / DK)  # [32]
    t_idx = np.arange(T, dtype=np.float64)
    ang = np.outer(t_idx, inv)                      # [T, 32]
    cos_t, sin_t = np.cos(ang), np.sin(ang)         # [T, 32]
    COSt = np.empty((128, 2, T), np.float32)
    SINt = np.empty((128, 2, T), np.float32)
    for b in range(2):
        for p in range(128):
            d = b * 128 + p
            dd = d % 64
            idx = dd % 32
            sign = 1.0 if dd < 32 else -1.0
            COSt[p, b, :] = cos_t[:, idx]
            SINt[p, b, :] = sign * sin_t[:, idx]

    # decay tables (chunk-invariant), feature-major [128, 2, C]
    DQt = np.empty((128, 2, C), np.float32)
    for b in range(2):
        for p in range(128):
            h = (b * 128 + p) // 64
            DQt[p, b, :] = gam[h] ** (i_idx + 1.0)
    # dkv token-major [128(j%128), 2(jb), C(dim col)]
    DKVt = np.empty((128, 2, C), np.float32)
    for jb in range(2):
        j = jb * 128 + np.arange(128, dtype=np.float64)
        for hcol in range(4):
            DKVt[:, jb, hcol * 64:(hcol + 1) * 64] = (
                gam[hcol] ** (C - 1.0 - j))[:, None]
    # Dmat^T [128(j%128), h, jb, C(i)]
    DMTt = np.zeros((128, HPC, 2, C), np.float32)
    for h in range(HPC):
        for jb in range(2):
            j = (jb * 128 + np.arange(128, dtype=np.float64))[:, None]
            rel = i_idx[None, :] - j
            DMTt[:, h, jb, :] = np.where(rel >= 0.0, gam[h] ** np.maximum(rel, 0.0), 0.0)
    # gamma^C per state-pair row
    GCVt = np.empty((128, 2), np.float32)
    for p in range(2):
        GCVt[0:64, p] = gam[2 * p] ** C
        GCVt[64:128, p] = gam[2 * p + 1] ** C
    return COSt, SINt, DQt, DKVt, DMTt, GCVt


def _prepare_inputs(x, Wq, Wk, Wv, Wg, Wo, g_norm_w):
    x = np.asarray(x, np.float32)
    Wq = np.asarray(Wq, np.float32) * (DK ** -0.5)
    Wk = np.asarray(Wk, np.float32)
    Wv = np.asarray(Wv, np.float32)
    Wg = np.asarray(Wg, np.float32)
    Wo = np.asarray(Wo, np.float32)
    gw = np.asarray(g_norm_w, np.float32)

    in_maps = []
    for core in range(NCORES):
        b = core // 4
        hg = core % 4
        heads = np.arange(4 * hg, 4 * hg + 4)
        qk_cols = np.concatenate([np.arange(h * DK, (h + 1) * DK) for h in heads])
        vg_cols = np.concatenate([np.arange(h * DV, (h + 1) * DV) for h in heads])

        XTc = np.ascontiguousarray(x[b].T).astype(BF)
        WQc = np.ascontiguousarray(
            Wq[:, qk_cols].reshape(8, 128, 256).transpose(1, 0, 2)).astype(BF)
        WKc = np.ascontiguousarray(
            Wk[:, qk_cols].reshape(8, 128, 256).transpose(1, 0, 2)).astype(BF)
        WVc = np.ascontiguousarray(
            Wv[:, vg_cols].reshape(8, 128, 512).transpose(1, 0, 2)).astype(BF)
        WGc = np.ascontiguousarray(
            Wg[:, vg_cols].reshape(8, 128, 512).transpose(1, 0, 2)).astype(BF)
        WOc = np.ascontiguousarray(
            Wo[vg_cols, :].reshape(4, 128, 1024).transpose(1, 0, 2)).astype(BF)

        COSt, SINt, DQt, DKVt, DMTt, GCVt = _host_tables(heads)
        CSt = np.concatenate([COSt, SINt], axis=1).astype(BF)

        in_maps.append({
            "XT": XTc, "WQ": WQc, "WK": WKc, "WV": WVc, "WG": WGc, "WO": WOc,
            "CS": CSt, "DQT": DQt.astype(BF), "DKVT": DKVt, "DMT": DMTt,
            "GCV": GCVt, "GW": np.ascontiguousarray(gw.reshape(128, 1)),
            "ONES": np.ones((128, 1), BF),
            "ONESR": np.ones((1, 128), BF),
            "IDENT": np.eye(128, dtype=BF),
            "ZS": np.zeros((128, 2, C), BF),
        })
    return in_maps


def _run(in_maps, **kw):
    if "nc" not in _cache:
        _cache["nc"] = _build_program()
    return run_bass_kernel_spmd(_cache["nc"], in_maps,
                                core_ids=list(range(NCORES)), **kw)


def kernel(x, Wq, Wk, Wv, Wg, Wo, g_norm_w):
    in_maps = _prepare_inputs(x, Wq, Wk, Wv, Wg, Wo, g_norm_w)
    res = _run(in_maps)
    out = np.zeros((B, T, D), np.float32)
    for core in range(NCORES):
        out[core // 4] += res.results[core]["OUT"]
    return out


# revision 28
# speedup vs baseline: 1.0326x; 1.0103x over previous
"""FLARetNet Trainium2 kernel: 8-core SPMD, batch x head-group sharding.

Each core handles one batch (B=2 -> 4 cores per batch) and 4 of 16 heads.
Per core: qkvg projections (fp16 matmuls), neox RoPE, RetNet chunked
retention scan (chunk=256), fused RMSNorm + swish gate, output projection
(partial sum over its heads). Host sums the 4 partials per batch.

All matmuls run in fp16 (full PE rate, fast weight load, ~10-bit
mantissa); PSUM accumulation is fp32. Decay/mask/rope tables are
host-precomputed inputs. The per-chunk Wo projection is deferred by one
chunk so the norm/gate chain stays off the PE critical path.
"""
import numpy as np
import ml_dtypes

import concourse.mybir as mybir
import concourse.tile as tile
import concourse.bacc as bacc
from concourse.bass_utils import run_bass_kernel_spmd

F32 = mybir.dt.float32
BF16 = mybir.dt.float16
AF = mybir.ActivationFunctionType
BF = np.float16

B, T, D, H = 2, 4096, 1024, 16
DK, DV = 64, 128
C = 256            # attention chunk length (math-equivalent for any C)
PT = 512           # projection token-tile
NCH = T // C       # 16 chunks
HPC = 4            # heads per core
NCORES = 8

_cache = {}


def _build_program():
    nc = bacc.Bacc("TRN2", target_bir_lowering=False, debug=False)

    XT = nc.dram_tensor("XT", [D, T], BF16, kind="ExternalInput")
    WQ = nc.dram_tensor("WQ", [128, 8, 256], BF16, kind="ExternalInput")
    WK = nc.dram_tensor("WK", [128, 8, 256], BF16, kind="ExternalInput")
    WV = nc.dram_tensor("WV", [128, 8, 512], BF16, kind="ExternalInput")
    WG = nc.dram_tensor("WG", [128, 8, 512], BF16, kind="ExternalInput")
    WO = nc.dram_tensor("WO", [128, 4, 1024], BF16, kind="ExternalInput")
    CS = nc.dram_tensor("CS", [128, 4, T], BF16, kind="ExternalInput")
    DQT = nc.dram_tensor("DQT", [128, 2, C], BF16, kind="ExternalInput")
    DKVT = nc.dram_tensor("DKVT", [128, 2, C], F32, kind="ExternalInput")
    DMT = nc.dram_tensor("DMT", [128, HPC, 2, C], F32, kind="ExternalInput")
    GCV = nc.dram_tensor("GCV", [128, 2], F32, kind="ExternalInput")
    GW = nc.dram_tensor("GW", [128, 1], F32, kind="ExternalInput")
    ONES = nc.dram_tensor("ONES", [128, 1], BF16, kind="ExternalInput")
    ONESR = nc.dram_tensor("ONESR", [1, 128], BF16, kind="ExternalInput")
    IDENT = nc.dram_tensor("IDENT", [128, 128], BF16, kind="ExternalInput")
    ZS = nc.dram_tensor("ZS", [128, 2, C], BF16, kind="ExternalInput")

    OUT = nc.dram_tensor("OUT", [T, D], F32, kind="ExternalOutput")

    with tile.TileContext(nc) as tc:
        with tc.tile_pool(name="singles", bufs=1) as singles, \
             tc.tile_pool(name="xt", bufs=2) as xt_pool, \
             tc.tile_pool(name="tab", bufs=2) as tab_pool, \
             tc.tile_pool(name="rope", bufs=2) as rope_pool, \
             tc.tile_pool(name="qk", bufs=2) as qk_pool, \
             tc.tile_pool(name="vsb", bufs=2) as v_pool, \
             tc.tile_pool(name="asb", bufs=3) as a_pool, \
             tc.tile_pool(name="gat", bufs=2) as g_pool, \
             tc.tile_pool(name="nrm", bufs=2) as nrm_pool, \
             tc.tile_pool(name="og", bufs=4) as og_pool, \
             tc.tile_pool(name="osb", bufs=3) as out_pool, \
             tc.tile_pool(name="ps_proj", bufs=3, space="PSUM") as ps_proj, \
             tc.tile_pool(name="ps_small", bufs=3, space="PSUM") as ps_small, \
             tc.tile_pool(name="ps_o", bufs=2, space="PSUM") as ps_o:

            # ---- resident weights/tables ----
            wq = singles.tile([128, 8, 256], BF16)
            wk = singles.tile([128, 8, 256], BF16)
            wv = singles.tile([128, 8, 512], BF16)
            wg = singles.tile([128, 8, 512], BF16)
            wo = singles.tile([128, 4, 1024], BF16)
            nc.gpsimd.dma_start(out=wq, in_=WQ[:, :, :])
            nc.gpsimd.dma_start(out=wk, in_=WK[:, :, :])
            nc.gpsimd.dma_start(out=wv, in_=WV[:, :, :])
            nc.gpsimd.dma_start(out=wg, in_=WG[:, :, :])
            nc.gpsimd.dma_start(out=wo, in_=WO[:, :, :])

            dqt = singles.tile([128, 2, C], BF16)
            dkvt = singles.tile([128, 2, C], F32)
            dmt = singles.tile([128, HPC, 2, C], F32)
            gcv = singles.tile([128, 2], F32)
            gwt = singles.tile([128, 1], F32)
            ones = singles.tile([128, 1], BF16)
            identmm = singles.tile([128, 128], BF16)
            nc.gpsimd.dma_start(out=identmm, in_=IDENT[:, :])
            nc.gpsimd.dma_start(out=dqt, in_=DQT[:, :, :])
            nc.gpsimd.dma_start(out=dkvt, in_=DKVT[:, :, :])
            nc.gpsimd.dma_start(out=dmt, in_=DMT[:, :, :, :])
            nc.gpsimd.dma_start(out=gcv, in_=GCV[:, :])
            nc.gpsimd.dma_start(out=gwt, in_=GW[:, :])
            nc.gpsimd.dma_start(out=ones, in_=ONES[:, :])
            onesr = singles.tile([1, 128], BF16)
            nc.gpsimd.dma_start(out=onesr, in_=ONESR[:, :])

            # persistent recurrent state, packed per head-pair:
            # S_sb[:, p, :]: rows (h%2)*64..+64, cols (h%2)*128..+128 hold S_h
            s_sb = singles.tile([128, 2, C], BF16)
            nc.gpsimd.dma_start(out=s_sb, in_=ZS[:, :, :])

            epsb = singles.tile([1, 1], F32)
            nc.vector.memset(epsb, 1e-5)

            def emit_wo(og_pair_list, oc0):
                for tb in range(2):
                    for nn in range(2):
                        out_ps = ps_small.tile([128, 512], F32, tag="small",
                                               name=f"wo{oc0}_{tb}_{nn}")
                        for h in range(HPC):
                            nc.tensor.matmul(
                                out_ps,
                                lhsT=og_pair_list[h // 2][
                                    :, h % 2, tb * 128:(tb + 1) * 128],
                                rhs=wo[:, h, nn * 512:(nn + 1) * 512],
                                start=(h == 0), stop=(h == HPC - 1))
                        out_sb = out_pool.tile([128, 512], F32, tag="outsb",
                                               name=f"wos{oc0}_{tb}_{nn}")
                        nc.scalar.copy(out_sb, out_ps)
                        nc.sync.dma_start(
                            out=OUT[oc0 + tb * 128:oc0 + (tb + 1) * 128,
                                    nn * 512:(nn + 1) * 512],
                            in_=out_sb)

            pending_wo = None

            for pt in range(T // PT):
                p0 = pt * PT

                xt = xt_pool.tile([128, 8, PT], BF16, tag="xt")
                xt_src = XT.rearrange("(db p) t -> p db t", p=128)
                nc.sync.dma_start(out=xt[:, 0:4, :],
                                  in_=xt_src[:, 0:4, p0:p0 + PT])
                nc.sync.dma_start(out=xt[:, 4:8, :],
                                  in_=xt_src[:, 4:8, p0:p0 + PT])

                cs = tab_pool.tile([128, 4, PT], BF16, tag="cs")
                nc.sync.dma_start(out=cs, in_=CS[:, :, p0:p0 + PT])
                cos = cs[:, 0:2, :]
                sin = cs[:, 2:4, :]

                # ---- projections over PT tokens ----
                # q, k feature-major [128(dim%128), blk, tok] + RoPE fused
                def proj_rope(w, tag):
                    out = qk_pool.tile([128, 2, PT], BF16, tag=tag,
                                       name=f"{tag}{pt}")
                    for m in range(2):
                        pps = ps_proj.tile([128, PT], F32, tag="proj",
                                           name=f"{tag}ps{pt}_{m}")
                        for db in range(8):
                            nc.tensor.matmul(
                                pps, lhsT=w[:, db, m * 128:(m + 1) * 128],
                                rhs=xt[:, db, :],
                                start=(db == 0), stop=(db == 7))
                        tcos = rope_pool.tile([128, PT], F32, tag="tcos")
                        tsin = rope_pool.tile([128, PT], BF16, tag="tsin")
                        rot = rope_pool.tile([128, PT], BF16, tag="rot")
                        nc.vector.tensor_mul(tcos, pps, cos[:, m, :])
                        nc.vector.tensor_mul(tsin, pps, sin[:, m, :])
                        for g0 in (0, 64):
                            nc.gpsimd.dma_start(out=rot[g0:g0 + 32, :],
                                                in_=tsin[g0 + 32:g0 + 64, :])
                            nc.gpsimd.dma_start(out=rot[g0 + 32:g0 + 64, :],
                                                in_=tsin[g0:g0 + 32, :])
                        nc.vector.tensor_add(out[:, m, :], tcos, rot)
                    return out

                q_sb = proj_rope(wq, "q")   # [128, 2, PT] fp16
                k_sb = proj_rope(wk, "k")

                qdq = qk_pool.tile([128, 2, PT], BF16, tag="qdq")
                nc.vector.tensor_mul(
                    qdq.rearrange('p a (c b) -> p a c b', b=C),
                    q_sb.rearrange('p a (c b) -> p a c b', b=C),
                    dqt[:, :, None, :].broadcast_to([128, 2, PT // C, C]))

                # v token-major [128(tok%128), tb, dim]
                v_sb = v_pool.tile([128, 4, 512], BF16, tag="v")
                for tb in range(4):
                    v_ps = ps_proj.tile([128, 512], F32, tag="proj",
                                        name=f"vps{pt}_{tb}")
                    for db in range(8):
                        nc.tensor.matmul(
                            v_ps,
                            lhsT=xt[:, db, tb * 128:(tb + 1) * 128],
                            rhs=wv[:, db, :],
                            start=(db == 0), stop=(db == 7))
                    nc.scalar.copy(v_sb[:, tb, :], v_ps)

                # g feature-major per head-block -> silu
                gsil = g_pool.tile([128, 4, PT], F32, tag="gsil")
                for m in range(4):
                    g_ps = ps_proj.tile([128, PT], F32, tag="proj",
                                        name=f"gps{pt}_{m}")
                    for db in range(8):
                        nc.tensor.matmul(
                            g_ps, lhsT=wg[:, db, m * 128:(m + 1) * 128],
                            rhs=xt[:, db, :],
                            start=(db == 0), stop=(db == 7))
                    nc.scalar.activation(gsil[:, m, :], g_ps, AF.Silu)

                # ---- per 256-chunk attention ----
                for cc in range(PT // C):
                    ch = pt * (PT // C) + cc
                    c0 = ch * C
                    qs = q_sb[:, :, cc * C:(cc + 1) * C]
                    ks = k_sb[:, :, cc * C:(cc + 1) * C]
                    qd = qdq[:, :, cc * C:(cc + 1) * C]
                    vtb0 = cc * 2

                    # k token-major + dkv scaling
                    ktm_ps = ps_small.tile([128, 2, C], BF16, tag="small",
                                           name=f"ktm{ch}")
                    for tb in range(2):
                        for b in range(2):
                            nc.tensor.transpose(
                                ktm_ps[:, tb, b * 128:(b + 1) * 128],
                                ks[:, b, tb * 128:(tb + 1) * 128],
                                identmm)
                    kdkv = qk_pool.tile([128, 2, C], BF16, tag="kdkv")
                    nc.vector.tensor_mul(kdkv, ktm_ps, dkvt)

                    o_ps_pairs = [ps_o.tile([128, 2, C], F32, tag="o",
                                            name=f"o_ps{ch}_{i}")
                                  for i in range(2)]
                    a_sbs = [None] * HPC

                    def emit_at(h):
                        blk, pb = h // 2, (h % 2) * 64
                        at_ps = ps_small.tile([128, 2, C], F32, tag="small",
                                              name=f"at{ch}_{h}")
                        for jb in range(2):
                            nc.tensor.matmul(
                                at_ps[:, jb, :],
                                lhsT=ks[pb:pb + 64, blk,
                                        jb * 128:(jb + 1) * 128],
                                rhs=qs[pb:pb + 64, blk, :],
                                start=True, stop=True)
                        a_sb = a_pool.tile([128, 2, C], BF16, tag="a",
                                           name=f"a{ch}_{h}")
                        nc.vector.tensor_mul(a_sb, at_ps, dmt[:, h, :, :])
                        a_sbs[h] = a_sb

                    def emit_o(h):
                        p, hh = h // 2, h % 2
                        blk, pb = h // 2, (h % 2) * 64
                        o_slice = o_ps_pairs[p][:, hh, :]
                        nc.tensor.matmul(
                            o_slice,
                            lhsT=s_sb[hh * 64:hh * 64 + 64, p,
                                      hh * 128:(hh + 1) * 128],
                            rhs=qd[pb:pb + 64, blk, :],
                            start=True, stop=False)
                        for jb in range(2):
                            nc.tensor.matmul(
                                o_slice,
                                lhsT=v_sb[:, vtb0 + jb, h * 128:(h + 1) * 128],
                                rhs=a_sbs[h][:, jb, :],
                                start=False, stop=(jb == 1))

                    emit_at(0)
                    for h in range(1, HPC):
                        emit_at(h)
                        emit_o(h - 1)

                    # deferred output projection of previous chunk: gives the
                    # previous norm chain time to finish off the PE critical path
                    if pending_wo is not None:
                        emit_wo(*pending_wo)
                        pending_wo = None

                    emit_o(HPC - 1)

                    # state update (packed per pair)
                    ds_ps = ps_small.tile([128, 2, C], F32, tag="small",
                                          name=f"ds{ch}")
                    for p in range(2):
                        for jb in range(2):
                            nc.tensor.matmul(
                                ds_ps[:, p, :],
                                lhsT=kdkv[:, jb, p * 128:(p + 1) * 128],
                                rhs=v_sb[:, vtb0 + jb, p * 256:(p + 1) * 256],
                                start=(jb == 0), stop=(jb == 1))
                    for p in range(2):
                        nc.vector.scalar_tensor_tensor(
                            out=s_sb[:, p, :],
                            in0=s_sb[:, p, :],
                            scalar=gcv[:, p:p + 1],
                            in1=ds_ps[:, p, :],
                            op0=mybir.AluOpType.mult,
                            op1=mybir.AluOpType.add)

                    # norm + gate per pair
                    og_pairs = []
                    for p in range(2):
                        o_ps = o_ps_pairs[p]
                        o_flat = o_ps.rearrange('p a b -> p (a b)')
                        osc = nrm_pool.tile([128, 512], F32, tag="osc",
                                            name=f"osc{ch}_{p}")
                        nc.scalar.copy(osc, o_flat)
                        o2 = nrm_pool.tile([128, 512], BF16, tag="o2",
                                           name=f"o2{ch}_{p}")
                        nc.vector.tensor_mul(o2, osc, o_flat)
                        mean_ps = ps_small.tile([1, 512], F32, tag="small",
                                                name=f"mean{ch}_{p}")
                        nc.tensor.matmul(mean_ps, lhsT=ones, rhs=o2,
                                         start=True, stop=True)
                        rsq1 = nrm_pool.tile([1, 512], F32, tag="rsq1",
                                             name=f"rsq{ch}_{p}")
                        nc.scalar.activation(rsq1, mean_ps,
                                             AF.Abs_reciprocal_sqrt,
                                             bias=epsb, scale=1.0 / DV)
                        bc = nrm_pool.tile([128, 512], F32, tag="bc",
                                           name=f"bcb{ch}_{p}")
                        nc.gpsimd.partition_broadcast(bc, rsq1)
                        onrm = nrm_pool.tile([128, 512], F32, tag="onrm",
                                             name=f"onrm{ch}_{p}")
                        nc.vector.tensor_mul(onrm, osc, bc)
                        og = og_pool.tile([128, 2, C], BF16, tag="og",
                                          name=f"og{ch}_{p}")
                        gs = gsil[:, p * 2:(p + 1) * 2, cc * C:(cc + 1) * C]
                        nc.vector.scalar_tensor_tensor(
                            out=og,
                            in0=onrm.rearrange('p (a b) -> p a b', a=2),
                            scalar=gwt, in1=gs,
                            op0=mybir.AluOpType.mult,
                            op1=mybir.AluOpType.mult)
                        og_pairs.append(og)

                    if ch == NCH - 1:
                        emit_wo(og_pairs, c0)
                    else:
                        pending_wo = (og_pairs, c0)

    nc.finalize()
    return nc


def _host_tables(heads):
    """Per-core constant tables for a 4-head slice."""
    gam = (1.0 - 2.0 ** (-5.0 - np.arange(H, dtype=np.float64)))[heads]  # [4]
    i_idx = np.arange(C, dtype=np.float64)

    # rope tables, feature-major [128, 2, T]
    inv = 10000.0 ** (-np.arange(0, DK, 2, dtype=np.float64) / DK)  # [32]
    t_idx = np.arange(T, dtype=np.float64)
    ang = np.outer(t_idx, inv)                      # [T, 32]
    cos_t, sin_t = np.cos(ang), np.sin(ang)         # [T, 32]
    COSt = np.empty((128, 2, T), np.float32)
    SINt = np.empty((128, 2, T), np.float32)
    for b in range(2):
        for p in range(128):
            d = b * 128 + p
            dd = d % 64
            idx = dd % 32
            sign = 1.0 if dd < 32 else -1.0
            COSt[p, b, :] = cos_t[:, idx]
            SINt[p, b, :] = sign * sin_t[:, idx]

    # decay tables (chunk-invariant), feature-major [128, 2, C]
    DQt = np.empty((128, 2, C), np.float32)
    for b in range(2):
        for p in range(128):
            h = (b * 128 + p) // 64
            DQt[p, b, :] = gam[h] ** (i_idx + 1.0)
    # dkv token-major [128(j%128), 2(jb), C(dim col)]
    DKVt = np.empty((128, 2, C), np.float32)
    for jb in range(2):
        j = jb * 128 + np.arange(128, dtype=np.float64)
        for hcol in range(4):
            DKVt[:, jb, hcol * 64:(hcol + 1) * 64] = (
                gam[hcol] ** (C - 1.0 - j))[:, None]
    # Dmat^T [128(j%128), h, jb, C(i)]
    DMTt = np.zeros((128, HPC, 2, C), np.float32)
    for h in range(HPC):
        for jb in range(2):
            j = (jb * 128 + np.arange(128, dtype=np.float64))[:, None]
            rel = i_idx[None, :] - j
            DMTt[:, h, jb, :] = np.where(rel >= 0.0, gam[h] ** np.maximum(rel, 0.0), 0.0)
    # gamma^C per state-pair row
    GCVt = np.empty((128, 2), np.float32)
    for p in range(2):
        GCVt[0:64, p] = gam[2 * p] ** C
        GCVt[64:128, p] = gam[2 * p + 1] ** C
    return COSt, SINt, DQt, DKVt, DMTt, GCVt


def _prepare_inputs(x, Wq, Wk, Wv, Wg, Wo, g_norm_w):
    x = np.asarray(x, np.float32)
    Wq = np.asarray(Wq, np.float32) * (DK ** -0.5)
    Wk = np.asarray(Wk, np.float32)
    Wv = np.asarray(Wv, np.float32)
    Wg = np.asarray(Wg, np.float32)
    Wo = np.asarray(Wo, np.float32)
    gw = np.asarray(g_norm_w, np.float32)

    in_maps = []
    for core in range(NCORES):
        b = core // 4
        hg = core % 4
        heads = np.arange(4 * hg, 4 * hg + 4)
        qk_cols = np.concatenate([np.arange(h * DK, (h + 1) * DK) for h in heads])
        vg_cols = np.concatenate([np.arange(h * DV, (h + 1) * DV) for h in heads])

        XTc = np.ascontiguousarray(x[b].T).astype(BF)
        WQc = np.ascontiguousarray(
            Wq[:, qk_cols].reshape(8, 128, 256).transpose(1, 0, 2)).astype(BF)
        WKc = np.ascontiguousarray(
            Wk[:, qk_cols].reshape(8, 128, 256).transpose(1, 0, 2)).astype(BF)
        WVc = np.ascontiguousarray(
            Wv[:, vg_cols].reshape(8, 128, 512).transpose(1, 0, 2)).astype(BF)
        WGc = np.ascontiguousarray(
            Wg[:, vg_cols].reshape(8, 128, 512).transpose(1, 0, 2)).astype(BF)
        WOc = np.ascontiguousarray(
            Wo[vg_cols, :].reshape(4, 128, 1024).transpose(1, 0, 2)).astype(BF)

        COSt, SINt, DQt, DKVt, DMTt, GCVt = _host_tables(heads)
        CSt = np.concatenate([COSt, SINt], axis=1).astype(BF)

        in_maps.append({
            "XT": XTc, "WQ": WQc, "WK": WKc, "WV": WVc, "WG": WGc, "WO": WOc,
            "CS": CSt, "DQT": DQt.astype(BF), "DKVT": DKVt, "DMT": DMTt,
            "GCV": GCVt, "GW": np.ascontiguousarray(gw.reshape(128, 1)),
            "ONES": np.ones((128, 1), BF),
            "ONESR": np.ones((1, 128), BF),
            "IDENT": np.eye(128, dtype=BF),
            "ZS": np.zeros((128, 2, C), BF),
        })
    return in_maps


def _run(in_maps, **kw):
    if "nc" not in _cache:
        _cache["nc"] = _build_program()
    return run_bass_kernel_spmd(_cache["nc"], in_maps,
                                core_ids=list(range(NCORES)), **kw)


def kernel(x, Wq, Wk, Wv, Wg, Wo, g_norm_w):
    in_maps = _prepare_inputs(x, Wq, Wk, Wv, Wg, Wo, g_norm_w)
    res = _run(in_maps)
    out = np.zeros((B, T, D), np.float32)
    for core in range(NCORES):
        out[core // 4] += res.results[core]["OUT"]
    return out
